# revision 31
# baseline (speedup 1.0000x reference)
"""Masked dot-product attention (B=16, LQ=LK=2048, D=64) on 8 TRN2 NeuronCores.

Strategy (final: per-k-tile pipeline, ACT+DVE+Pool exp split, bf16/fp16 data)
-----------------------------------------------------------------------------
out[b] = softmax(mask(Q K^T / 8)) V, keys >= valid_len[b] masked.

Each (batch, 512-query quarter) job is one segment of ceil(valid_len/128)
k-tiles; segments are sorted and dealt 8-at-a-time into 8 slot ranks so all
cores run one instruction stream (rank_lens = compiled per-slot lengths).

Per k-tile (flat stream across slots, 6-deep 1-bank PSUM score ring):
  MM1   S^T[kk, q] = (K^T tile).T @ Q^T    bf16 in, f32 PSUM [128,512]
  EXP   P = exp(0.125 S^T) -> fp16 SBUF, engine chosen by (2s+kt)%3:
    2/3 of tiles: exact table exp on the scalar engine (ACT).
    1/3 of tiles: 2-sawtooth-sum approx (max shape err 1.06%; its global
      gain is compensated at pack time by scaling those tiles' V+ones
      columns by 1/SW_GAIN):
        b1 = trunc_i16(S * 1024*log2e/8 + B1)   (DVE tensor_scalar, 1x)
        b2 = b1 + D                             (DVE int16, 4x mode)
        P  = fp16(b1) + fp16(b2)                (Pool tensor_tensor add)
  MM2   acc[q, 65*qc+d | 64] += P_chunk.T @ [V|ones]   (fp16 in, f32 PSUM)
    4 q-chunks of 128, out free 65 -> 1 bf16-cycle/row on the PE.

acc is ONE [128,512] PSUM bank per slot (4 q-chunks x 65 cols), accumulated
over the whole segment (start flag on first MM2, stop on last; only the
first matmul per bank may set start -- the 2KB zero-region is bank-wide).
Masking via zeroed V rows + ones-column.  Epilogue: one [128,260] DVE copy
+ DMA per slot; host sums segments and divides (output is already [q, d]).

Schedule notes (all measured in the graded cost-model timeline): shortest
slots first with the longest second-to-last; whole-job segments (8 slots)
beat split segments; DVE share above 1/3 or any adjacent DVE tiles backs up
the score ring and loses; bf16 Q/K halves the startup DMA.  Engine busy:
ACT ~27us, DVE ~21us, Pool ~13us, PE ~23us in a ~37us envelope.
"""

import math
from contextlib import ExitStack

import numpy as np

import concourse.bacc as bacc
import concourse.mybir as mybir
import concourse.tile as tile
import concourse.bass_utils as bass_utils

B, LQ, LK, D = 16, 2048, 2048, 64
N_CORES = 8
KT = 128          # keys per k-tile
QS = 512          # queries per slot (q-quarter)
SEG = 16          # max k-tiles per segment (16 = whole jobs)
SCALE = 1.0 / math.sqrt(D)

F32 = mybir.dt.float32
F16 = mybir.dt.float16
I16 = mybir.dt.int16
MM_DT = mybir.dt.bfloat16

# SW2 approx-exp constants (p ~= fp16bits(b1) + fp16bits(b1+D), fit err 1.06%)
C16 = 1024 * 1.4426950408889634 / 8.0
SW_B1 = 15712.0
SW_D = -496.0
SW_GAIN = 2.2533878635239586
def tail_slot(rank_lens):
    """Index of the slot that executes last (mirrors build_bass's order)."""
    slots = len(rank_lens)
    asc = sorted(range(slots), key=lambda s: rank_lens[s])
    order = asc[:-2] + [asc[-1], asc[-2]] if slots > 2 else asc
    return order[-1]


def dve_unit(s, u, rank_lens=None):
    """Static engine assignment for k-tile u of slot s (shared with packing)."""
    return ((2 * s + u) % 3) == 1


def pair_layout(rank_lens, j):
    """Column offsets inside the pair's qk tensor (q | k sections)."""
    na = rank_lens[2 * j]
    qo = 0
    ko = qo + QS
    width = ko + na * KT
    return qo, ko, width


def build_bass(rank_lens, cfg=None):
    cf = {"sp": 6, "pp": 10, "ep": 4, "op3_pool": 1, "copy_act": 0}
    if cfg:
        cf.update(cfg)
    slots = len(rank_lens)
    pairs = slots // 2
    nc = bacc.Bacc("TRN2", target_bir_lowering=False, debug=False)

    widths = [pair_layout(rank_lens, j)[2] for j in range(pairs)]
    vw = [(rank_lens[2 * j] + rank_lens[2 * j + 1]) * (D + 1) for j in range(pairs)]
    pk = [
        nc.dram_tensor(f"pk{j}", [128, widths[j]], MM_DT, kind="ExternalInput").ap()
        for j in range(pairs)
    ]
    pv = [
        nc.dram_tensor(f"pv{j}", [128, vw[j]], F16, kind="ExternalInput").ap()
        for j in range(pairs)
    ]
    out = nc.dram_tensor("out", [slots * 128, 260], F16, kind="ExternalOutput").ap()

    Exp = mybir.ActivationFunctionType.Exp
    Mult = mybir.AluOpType.mult
    Add = mybir.AluOpType.add

    with tile.TileContext(nc) as tc, ExitStack() as ctx:
        inp = ctx.enter_context(tc.tile_pool(name="inp", bufs=1))
        ppool = ctx.enter_context(tc.tile_pool(name="pp", bufs=cf["pp"]))
        bpool = ctx.enter_context(tc.tile_pool(name="bp", bufs=cf.get("bp", 3)))
        epool = ctx.enter_context(tc.tile_pool(name="ep", bufs=cf["ep"]))
        spool = ctx.enter_context(tc.tile_pool(name="sp", bufs=cf["sp"], space="PSUM"))
        apool = ctx.enter_context(tc.tile_pool(name="ap", bufs=2, space="PSUM"))

        asc = sorted(range(slots), key=lambda s: rank_lens[s])
        # shortest slots first (fast DMA startup); longest slot second-to-last
        # so the very last slot's epilogue tail is short
        order = cf.get("order") or (asc[:-2] + [asc[-1], asc[-2]] if slots > 2 else asc)
        pair_order = sorted(range(pairs), key=lambda j: rank_lens[2 * j])

        qk_t = [None] * pairs
        km_t = [None] * pairs   # k-tiles [2, nb)
        kx_t = [None] * pairs   # k-tiles [nb, na) (longer slot's overflow)
        kx_at = [None] * pairs
        v_t = [None] * pairs
        # Gather DMA thunks with phases, then emit in phase order: the first
        # pairs' q+first-k-tiles and V go before any pair's k middle/tail so
        # early compute is never queued behind bulk transfers.
        thunks = []  # (phase, emit_fn)

        def add_qk(j, rank):
            na, nb = rank_lens[2 * j], rank_lens[2 * j + 1]
            split1 = min(2, nb)
            if j in pair_order[:2] and na > split1:
                w1 = QS + split1 * KT
                qk_t[j] = inp.tile([128, w1], MM_DT, name=f"qk{j}")
                thunks.append((rank, lambda j=j, w1=w1: nc.sync.dma_start(qk_t[j][:], pk[j][:, :w1])))
                if nb > split1:
                    km_t[j] = inp.tile([128, (nb - split1) * KT], MM_DT, name=f"km{j}")
                    thunks.append((rank + 2, lambda j=j, w1=w1, nb=nb: nc.sync.dma_start(
                        km_t[j][:], pk[j][:, w1 : QS + nb * KT])))
                if na > nb:
                    kx_t[j] = inp.tile([128, (na - nb) * KT], MM_DT, name=f"kx{j}")
                    kx_at[j] = nb
                    thunks.append((rank + 3, lambda j=j, nb=nb: nc.sync.dma_start(
                        kx_t[j][:], pk[j][:, QS + nb * KT : widths[j]])))
            elif na > nb:
                wa = QS + nb * KT
                qk_t[j] = inp.tile([128, wa], MM_DT, name=f"qk{j}")
                thunks.append((rank, lambda j=j, wa=wa: nc.sync.dma_start(qk_t[j][:], pk[j][:, :wa])))
                kx_t[j] = inp.tile([128, (na - nb) * KT], MM_DT, name=f"kx{j}")
                kx_at[j] = nb
                thunks.append((rank + 3, lambda j=j, wa=wa: nc.sync.dma_start(
                    kx_t[j][:], pk[j][:, wa : widths[j]])))
            else:
                qk_t[j] = inp.tile([128, widths[j]], MM_DT, name=f"qk{j}")
                thunks.append((rank, lambda j=j: nc.sync.dma_start(qk_t[j][:], pk[j][:, : widths[j]])))

        for i, j in enumerate(pair_order):
            base = i * 10 if i >= 2 else i * 2
            add_qk(j, base)
            v_t[j] = inp.tile([128, vw[j]], F16, name=f"v{j}")
            veng = nc.gpsimd if i < 2 and cf.get("mq", 0) else nc.sync
            thunks.append((base + 1, lambda j=j, e=veng: e.dma_start(v_t[j][:], pv[j][:, :])))
        for _, emit in sorted(thunks, key=lambda t: t[0]):
            emit()

        def k_lhsT(j, pb, kt):
            if kx_at[j] is not None and kt >= kx_at[j]:
                kk = kt - kx_at[j]
                return kx_t[j][pb : pb + 64, kk * KT : (kk + 1) * KT]
            if km_t[j] is not None and kt >= 2 and qk_t[j].shape[1] <= QS + 2 * KT:
                return km_t[j][pb : pb + 64, (kt - 2) * KT : (kt - 1) * KT]
            ko = QS
            return qk_t[j][pb : pb + 64, ko + kt * KT : ko + (kt + 1) * KT]

        # Flat unit stream across all slots.  Each unit emits MM1 + exp at
        # its turn; its MM2 batch is deferred LAG units (longer for the
        # higher-latency DVE chain) so the in-order PE never head-of-line
        # blocks on a not-yet-computed P tile.
        # Flat per-k-tile stream across all slots: one 1-bank [128,512] PSUM
        # score tile per k-tile gives a 6-deep ring (vs 3 for 2-bank tiles),
        # which is what keeps MM1 from stalling on exp-queue latency.
        op3_pool = cf.get("op3_pool", 0)
        stream = []
        slot_state = {}
        for s in order:
            ns = rank_lens[s]
            slot_state[s] = {"emitted": 0, "ns": ns, "acc": None}
            for kt in range(ns):
                stream.append((s, kt))
        if cf.get("ilv_tail", 0) and slots > 2:
            # Interleave the last two slots' tiles: the end-game gets two
            # independent dependency chains to overlap (2 accs = apool bufs).
            sa, sb = order[-2], order[-1]
            na_, nb_ = rank_lens[sa], rank_lens[sb]
            head = stream[: -(na_ + nb_)]
            a = [(sa, k) for k in range(na_)]
            b = [(sb, k) for k in range(nb_)]
            mix = []
            while a or b:
                if len(a) >= len(b) and a:
                    mix.append(a.pop(0))
                elif b:
                    mix.append(b.pop(0))
            stream = head + mix

        last_dve = None
        for gi, (s, kt) in enumerate(stream):
            if dve_unit(s, kt, rank_lens):
                last_dve = gi
        dcnt = 0
        for gi, (s, kt) in enumerate(stream):
            ns = rank_lens[s]
            j = s // 2
            pb = (s % 2) * 64
            pt = qk_t[j]
            st = slot_state[s]
            if st["acc"] is None:
                st["acc"] = apool.tile([128, 512], F32, name=f"acc{s}", tag="acc")
            s_ps = spool.tile([128, QS], F32, name="s_ps")
            nc.tensor.matmul(
                s_ps[:, :],
                k_lhsT(j, pb, kt),
                pt[pb : pb + 64, 0:QS],
                start=True,
                stop=True,
            )
            p_t = ppool.tile([128, QS], F16, name="p_t")
            if dve_unit(s, kt, rank_lens):
                b1 = bpool.tile([128, QS], I16, name="b1")
                nc.vector.tensor_scalar(b1[:], s_ps[:], C16, SW_B1, Mult, Add)
                b2 = bpool.tile([128, QS], I16, name="b2")
                nc.vector.tensor_scalar(b2[:], b1[:], SW_D, None, Add)
                dcnt += 1
                eng = (
                    nc.vector
                    if gi == last_dve
                    else (nc.gpsimd if (op3_pool and dcnt % op3_pool == 0) else nc.vector)
                )
                eng.tensor_tensor(
                    p_t[:], b1[:].bitcast(F16), b2[:].bitcast(F16), Add
                )
            else:
                nc.scalar.activation(p_t[:], s_ps[:], Exp, scale=SCALE)
            voff = (s % 2) * rank_lens[2 * j] * (D + 1)
            wv = v_t[j][:, voff + kt * (D + 1) : voff + (kt + 1) * (D + 1)]
            first = st["emitted"] == 0
            st["emitted"] += 1
            last_batch = st["emitted"] == ns
            for qc in range(4):
                nc.tensor.matmul(
                    st["acc"][:, qc * 65 : qc * 65 + 65],
                    p_t[:, qc * 128 : (qc + 1) * 128],
                    wv,
                    start=(first and qc == 0),
                    stop=(last_batch and qc == 3),
                )
            if last_batch:
                acc_sb = epool.tile([128, 260], F16, name="acc_sb")
                ca = cf.get("copy_act", 2)
                if s == tail_slot(rank_lens) or (ca and s % ca == 1):
                    nc.scalar.copy(acc_sb[:], st["acc"][:, :260])
                else:
                    nc.vector.tensor_copy(acc_sb[:], st["acc"][:, :260])
                deng = nc.gpsimd if (s == tail_slot(rank_lens) and cf.get("tail_dma_pool", 0)) else nc.sync
                deng.dma_start(out[s * 128 : (s + 1) * 128, :], acc_sb[:])

    nc.compile()
    return nc


def plan_and_pack(queries, keys, values, valid_lens):
    """Split jobs into k-segments, deal into rank slots, gather inputs."""
    import ml_dtypes

    q = np.ascontiguousarray(np.asarray(queries, dtype=np.float32)).astype(
        ml_dtypes.bfloat16
    )
    k = np.asarray(keys, dtype=np.float32).astype(ml_dtypes.bfloat16)
    v = np.asarray(values, dtype=np.float32)
    vl = np.asarray(valid_lens, dtype=np.int64)

    nkt = np.maximum(1, -(-vl // KT))

    def make_segs(seg_max):
        segs = []  # (len_ktiles, b, qh, k0)
        for b in range(B):
            n = int(nkt[b])
            m = -(-n // seg_max)
            base, rem = divmod(n, m)
            sizes = [base + 1] * rem + [base] * (m - rem)
            for qh in range(LQ // QS):
                k0 = 0
                for sz in sizes:
                    segs.append((sz, b, qh, k0))
                    k0 += sz
        segs.sort(key=lambda t: (-t[0], t[1], t[2], t[3]))
        return segs

    def cost(segs):
        ls = sorted((s[0] for s in segs), reverse=True)
        while len(ls) % N_CORES:
            ls.append(0)
        slots = len(ls) // N_CORES
        if slots % 2:
            slots += 1
            ls += [0] * N_CORES
        rsum = sum(max(ls[N_CORES * r], 1) for r in range(slots))
        return rsum * 0.62 + slots * 0.8

    seg_best = min(range(4, SEG + 1), key=lambda m: cost(make_segs(m)))
    segs = make_segs(seg_best)
    while len(segs) % N_CORES:
        segs.append(None)
    slots = len(segs) // N_CORES
    if slots % 2:
        segs.extend([None] * N_CORES)
        slots += 1
    rank_lens = []
    for r in range(slots):
        first = segs[N_CORES * r]
        rank_lens.append(first[0] if first is not None else 1)
    pairs = slots // 2

    kT = np.swapaxes(k, 1, 2)
    parts = np.arange(KT)

    in_maps = []
    slot_map = []
    for c in range(N_CORES):
        core_map = {}
        smap = []
        for j in range(pairs):
            qo, ko, width = pair_layout(rank_lens, j)
            na = rank_lens[2 * j]
            pkj = np.zeros((128, width), dtype=ml_dtypes.bfloat16)
            pvj = np.zeros(
                (128, (na + rank_lens[2 * j + 1]) * (D + 1)), dtype=np.float16
            )
            for i, s in enumerate((2 * j, 2 * j + 1)):
                nr = rank_lens[s]
                seg = segs[N_CORES * s + c]
                if seg is None:
                    smap.append(None)
                    continue
                sz, b, qh, k0 = seg
                pb = i * 64
                smap.append((b, qh, k0))
                pkj[pb : pb + 64, qo : qo + QS] = q[b, qh * QS : (qh + 1) * QS, :].T
                kw = min(nr * KT, LK - k0 * KT)
                pkj[pb : pb + 64, ko : ko + kw] = kT[b, :, k0 * KT : k0 * KT + kw]
                voff = i * na * (D + 1)
                nv = kw // KT
                vs32 = np.zeros((128, nr, D + 1), dtype=np.float32)
                vs32[:, :nv, :D] = (
                    v[b, k0 * KT : k0 * KT + nv * KT, :]
                    .reshape(nv, KT, D)
                    .transpose(1, 0, 2)
                )
                vs32[:, :, D] = 1.0
                kid = (k0 + np.arange(nr))[None, :] * KT + parts[:, None]
                dead = (kid >= vl[b]) | (kid >= (k0 + sz) * KT)
                vs32[dead] = 0.0
                for kt in range(nr):
                    if dve_unit(s, kt, rank_lens):
                        vs32[:, kt, :] *= 1.0 / SW_GAIN
                pvj[:, voff : voff + nr * (D + 1)] = vs32.reshape(
                    128, nr * (D + 1)
                ).astype(np.float16)
            core_map[f"pk{j}"] = pkj
            core_map[f"pv{j}"] = pvj
        in_maps.append(core_map)
        slot_map.append(smap)
    return rank_lens, in_maps, slot_map


def scatter_out(results, slot_map):
    num = {}
    for c in range(N_CORES):
        oc = results[c]["out"]
        for s, seg in enumerate(slot_map[c]):
            if seg is None:
                continue
            b, qh, _ = seg
            blk = oc[s * 128 : (s + 1) * 128, :].astype(np.float64)
            key = (b, qh)
            if key in num:
                num[key] += blk
            else:
                num[key] = blk
    out = np.empty((B, LQ, D), dtype=np.float32)
    for (b, qh), a in num.items():
        a4 = a.reshape(128, 4, 65)
        res = a4[:, :, :D] / a4[:, :, D : D + 1]  # [128q, 4qc, D]
        out[b, qh * QS : (qh + 1) * QS, :] = res.transpose(1, 0, 2).reshape(QS, D)
    return out


def kernel(queries, keys, values, valid_lens, _run=None):
    rank_lens, in_maps, slot_map = plan_and_pack(queries, keys, values, valid_lens)
    nc = build_bass(rank_lens)
    if _run is not None:
        results = _run(nc, in_maps)
    else:
        import time as _time

        last = None
        for attempt in range(4):
            try:
                results = bass_utils.run_bass_kernel_spmd(
                    nc, in_maps, core_ids=list(range(N_CORES))
                ).results
                break
            except Exception as e:  # noqa: BLE001
                last = e
                _time.sleep(45.0 * (attempt + 1))
        else:
            raise last
    return scatter_out(results, slot_map)
